# revision 3
# baseline (speedup 1.0000x reference)
"""Trainium2 Bass kernel for nn_FuzzyMultiLayer.

Reference math (per point x in R^32, K=8 classes):
    L_k = tril(scale_k); z = L_k^{-1} (x - mu_k); maha_k = ||z||^2
    log_prob_k = -0.5*maha_k - 0.5*C*log(2pi) - log|det L_k|
    prob = exp(log_prob); g = prob * rsqrt(max(sum_k prob^2, 1e-12))
    out[.., k*C + c] = g_k * x_c

Key simplification: 0.5*C*log(2pi) = 29.43 with C=32, so prob_k <=
exp(1.65 - 29.44) ~ 9e-13 and sum(prob^2) <= 6e-24 << 1e-12 ALWAYS.
The max() floor therefore always selects 1e-12 and
    g_k = 1e6 * prob_k = exp(-0.5*maha_k + const_k),
    const_k = log(1e6) - 0.5*C*log(2pi) - logdet_k
No cross-class normalization is needed.

Sharding: pure data parallel, batch b -> core b (B == 8 == n_cores).
Per-core: x [65536, 32] -> out [65536, 256].

Pipeline per 512-point macro-tile (points n0..n0+511; point n0+4p+j lives
at SBUF partition p, slot j):
  1. DMA x tile X[128, 128]            (X[p, 32j+c] = x[n0+4p+j, c])
  2. 4x PE transpose -> xT psum [32, 512], ACT copy -> SBUF
  3. 2x PE matmul  z[cg] = LtT[cg].T @ xT   [128 class-chans, 512]
  4. ACT Square(z - v) per-partition bias -> u[cg] SBUF
  5. 2x PE matmul (accum) maha = mask.T @ u  [8, 512]
  6. ACT Exp(-0.5*maha + const_k) per-partition bias -> g [8, 512]
  7. 4x PE transpose g -> gT psum [128, 32]
  8. DVE broadcast multiply out[p, 256j + 32k + c] = gT[p, 8j+k]*X[p, 32j+c]
  9. DMA out [128, 1024]
"""

import math
import os
from contextlib import ExitStack

import numpy as np

import concourse.bacc as bacc
import concourse.tile as tile
from concourse import mybir
from concourse.bass_utils import run_bass_kernel_spmd

# Problem dims (hardcoded per contract)
B, H, W, C, K = 8, 256, 256, 32, 8
N = H * W          # points per core (one batch element per core)
N_CORES = 8
PTS = 512          # points per macro-tile
NMAC = N // PTS    # 128 macro-tiles
F32 = mybir.dt.float32

_BUILD_CACHE: dict = {}


def _build_nc(mm_dtype=mybir.dt.float32r):
    """Build + compile the SPMD Bass program (one NeuronCore's view)."""
    nc = bacc.Bacc("TRN2", target_bir_lowering=False, debug=False,
                   num_devices=N_CORES)

    MMDT = mm_dtype
    x_in = nc.dram_tensor("x", [N, C], F32, kind="ExternalInput").ap()
    lt_in = nc.dram_tensor("lt", [C, 2 * 128], MMDT, kind="ExternalInput").ap()
    negv_in = nc.dram_tensor("negv", [128, 2], F32, kind="ExternalInput").ap()
    kc_in = nc.dram_tensor("kc", [K, 1], F32, kind="ExternalInput").ap()
    mask_in = nc.dram_tensor("mask", [128, 16], MMDT, kind="ExternalInput").ap()
    id_in = nc.dram_tensor("ident", [128, 128], F32, kind="ExternalInput").ap()
    out_dram = nc.dram_tensor("out", [N, K * C], F32, kind="ExternalOutput").ap()

    with tile.TileContext(nc) as tc, ExitStack() as ctx:
        const = ctx.enter_context(tc.tile_pool(name="const", bufs=1))
        lt_sb = const.tile([C, 2 * 128], MMDT)
        nc.sync.dma_start(lt_sb[:], lt_in[:])
        negv_sb = const.tile([128, 2], F32)
        nc.sync.dma_start(negv_sb[:], negv_in[:])
        kc_sb = const.tile([K, 1], F32)
        nc.sync.dma_start(kc_sb[:], kc_in[:])
        mask_sb = const.tile([128, 16], MMDT)
        nc.sync.dma_start(mask_sb[:], mask_in[:])
        id_sb = const.tile([128, 128], F32)
        nc.sync.dma_start(id_sb[:], id_in[:])

        xp = ctx.enter_context(tc.tile_pool(name="xp", bufs=3))
        xt_pool = ctx.enter_context(tc.tile_pool(name="xt_ps", bufs=2, space="PSUM"))
        xt_sb_pool = ctx.enter_context(tc.tile_pool(name="xt_sb", bufs=2))
        z_pool = ctx.enter_context(tc.tile_pool(name="z_ps", bufs=2, space="PSUM"))
        u_pool = ctx.enter_context(tc.tile_pool(name="u_sb", bufs=2))
        maha_pool = ctx.enter_context(tc.tile_pool(name="maha_ps", bufs=2, space="PSUM"))
        g_pool = ctx.enter_context(tc.tile_pool(name="g_sb", bufs=2))
        gt_pool = ctx.enter_context(tc.tile_pool(name="gt_ps", bufs=2, space="PSUM"))
        out_pool = ctx.enter_context(tc.tile_pool(name="out_sb", bufs=3))

        for m in range(NMAC):
            n0 = m * PTS
            # 1. load X[p, 32j + c] = x[n0 + 4p + j, c]
            X = xp.tile([128, 128], F32)
            src = x_in[n0:n0 + PTS, :].rearrange("(p j) c -> p (j c)", j=4)
            nc.sync.dma_start(X[:], src)

            # 2. transpose -> xT[c, 128j + p] = X[p, 32j + c]
            xt_ps = xt_pool.tile([C, PTS], F32)
            for j in range(4):
                nc.tensor.transpose(
                    xt_ps[:, 128 * j:128 * (j + 1)],
                    X[:, 32 * j:32 * (j + 1)],
                    id_sb[:],
                )
            xt = xt_sb_pool.tile([C, PTS], MMDT)
            nc.scalar.copy(xt[:], xt_ps[:])

            # 3./4. z = LtT.T @ xT ; u = (z - v)^2
            us = []
            for cg in range(2):
                z_ps = z_pool.tile([128, PTS], F32)
                nc.tensor.matmul(
                    z_ps[:],
                    lt_sb[:, 128 * cg:128 * (cg + 1)],
                    xt[:],
                    start=True, stop=True,
                )
                u = u_pool.tile([128, PTS], MMDT)
                nc.scalar.activation(
                    u[:], z_ps[:], mybir.ActivationFunctionType.Square,
                    bias=negv_sb[:, cg:cg + 1], scale=1.0,
                )
                us.append(u)

            # 5. maha[k, f] = sum_cc mask[cc, k] * u[cc, f]  (accumulate 2 cgs)
            maha_ps = maha_pool.tile([K, PTS], F32)
            nc.tensor.matmul(
                maha_ps[:], mask_sb[:, 0:8],
                us[0][:], start=True, stop=False,
            )
            nc.tensor.matmul(
                maha_ps[:], mask_sb[:, 8:16],
                us[1][:], start=False, stop=True,
            )

            # 6. g = exp(-0.5*maha + const_k)
            g = g_pool.tile([K, PTS], F32)
            nc.scalar.activation(
                g[:], maha_ps[:], mybir.ActivationFunctionType.Exp,
                bias=kc_sb[:], scale=-0.5,
            )

            # 7. gT[p, 8q + k] = g[k, 128q + p]
            gt_ps = gt_pool.tile([128, 4 * K], F32)
            for q in range(4):
                nc.tensor.transpose(
                    gt_ps[:, 8 * q:8 * (q + 1)],
                    g[:, 128 * q:128 * (q + 1)],
                    id_sb[0:K, 0:K],
                )

            # 8. out[p, 256j + 32k + c] = gT[p, 8j + k] * X[p, 32j + c]
            out_sb = out_pool.tile([128, 4 * K * C], F32)
            o_ap = out_sb[:].rearrange("p (j k c) -> p j k c", j=4, k=K)
            x_ap = (X[:].rearrange("p (j c) -> p j c", j=4)
                    .unsqueeze(2).broadcast_to([128, 4, K, C]))
            g_ap = (gt_ps[:].rearrange("p (j k) -> p j k", j=4)
                    .unsqueeze(3).broadcast_to([128, 4, K, C]))
            nc.vector.tensor_mul(o_ap, g_ap, x_ap)

            # 9. store
            dst = out_dram[n0:n0 + PTS, :].rearrange("(p j) c -> p (j c)", j=4)
            nc.sync.dma_start(dst, out_sb[:])

    nc.compile()
    return nc


def _host_constants(mean: np.ndarray, scale: np.ndarray):
    """Precompute the tiny per-class parameter transforms on host."""
    L = np.tril(scale.astype(np.float64))                       # [K, C, C]
    eye = np.eye(C, dtype=np.float64)
    Linv = np.stack([np.linalg.solve(L[k], eye) for k in range(K)])  # [K, C, C]
    v = np.einsum("kcd,kd->kc", Linv, mean.astype(np.float64))  # [K, C]
    logdet = np.log(np.abs(np.diagonal(L, axis1=-2, axis2=-1))).sum(-1)  # [K]
    kconst = math.log(1e6) - 0.5 * C * math.log(2.0 * math.pi) - logdet  # [K]

    lt = np.zeros((C, 2 * 128), dtype=np.float32)
    negv = np.zeros((128, 2), dtype=np.float32)
    for k in range(K):
        cg, kk = divmod(k, 4)
        lt[:, 128 * cg + 32 * kk: 128 * cg + 32 * (kk + 1)] = Linv[k].T
        negv[32 * kk:32 * (kk + 1), cg] = -v[k]
    mask = np.zeros((128, 16), dtype=np.float32)
    for k in range(K):
        cg, kk = divmod(k, 4)
        mask[32 * kk:32 * (kk + 1), 8 * cg + k] = 1.0
    ident = np.eye(128, dtype=np.float32)
    return {
        "lt": lt,
        "negv": negv,
        "kc": kconst.astype(np.float32).reshape(K, 1),
        "mask": mask,
        "ident": ident,
    }


def _mm_dtype():
    name = os.environ.get("FUZZY_MM_DTYPE", "float32r")
    return getattr(mybir.dt, name)


def kernel(x: np.ndarray, mean: np.ndarray, scale: np.ndarray,
           _trace: bool = False) -> np.ndarray:
    assert x.shape == (B, H, W, C)
    key = ("nc", _mm_dtype())
    if key not in _BUILD_CACHE:
        _BUILD_CACHE[key] = _build_nc(_mm_dtype())
    nc = _BUILD_CACHE[key]

    consts = _host_constants(mean, scale)
    in_maps = []
    for b in range(N_CORES):
        m = {"x": np.ascontiguousarray(x[b].reshape(N, C), dtype=np.float32)}
        m.update(consts)
        in_maps.append(m)

    res = run_bass_kernel_spmd(nc, in_maps, list(range(N_CORES)), trace=_trace)
    if _trace:
        _BUILD_CACHE["last_exec_time_ns"] = res.exec_time_ns
        _BUILD_CACHE["last_profile"] = res.profile_json
    out = np.stack([res.results[b]["out"].reshape(H, W, K * C)
                    for b in range(N_CORES)])
    return out.astype(np.float32)
